# revision 1
# baseline (speedup 1.0000x reference)
import numpy as np

import concourse.bass as bass
import concourse.tile as tile
from concourse import bacc, mybir
from concourse.bass_utils import run_bass_kernel_spmd

E, H, D = 128, 8, 16
QDIM, DYN, HID = 16, 3, 64
ALPHA = 1.0
CLIP = 10.0
NCORES = 8
B, P, N = 16, 100, 1000
BPC = B // NCORES  # batches per core


def _np_softplus(x):
    # stable softplus, matches jax.nn.softplus in f32
    return np.log1p(np.exp(-np.abs(x))).astype(np.float32) + np.maximum(x, 0.0)


def _host_mh_out(encoded_nodes, encoded_last_node, load, ninf_mask, dyn_features,
                 Wq_last, Wk, Wv, W_comb, b_comb,
                 film_W1, film_b1, film_W2, film_b2,
                 lazy_q_W, lazy_q_b, lm_W1, lm_b1, lm_W2, lm_b2, lm_W3, lm_b3):
    """Everything up to mh_out, f32 numpy (sharded slice)."""
    b, n, _ = encoded_nodes.shape
    p = encoded_last_node.shape[1]
    k = (encoded_nodes @ Wk).reshape(b, n, H, D).transpose(0, 2, 1, 3)
    v = (encoded_nodes @ Wv).reshape(b, n, H, D).transpose(0, 2, 1, 3)
    q_in = np.concatenate([encoded_last_node, load[:, :, None]], axis=-1).astype(np.float32)
    q = (q_in @ Wq_last).reshape(b, p, H, D).transpose(0, 2, 1, 3)
    g = np.maximum(q_in @ film_W1 + film_b1, 0.0) @ film_W2 + film_b2
    gamma = (2.0 / (1.0 + np.exp(-g))).astype(np.float32).reshape(b, p, H, D).transpose(0, 2, 1, 3)
    q = q * gamma
    qf = q_in @ lazy_q_W + lazy_q_b
    # lazy mask MLP; fold qf part of layer 1 in as a per-(b,p) bias
    c1 = qf @ lm_W1[DYN:] + lm_b1                        # (b,p,HID)
    h = np.maximum(np.einsum('bpnd,dh->bpnh', dyn_features, lm_W1[:DYN],
                             dtype=np.float32) + c1[:, :, None, :], 0.0)
    h = np.maximum(h @ lm_W2 + lm_b2, 0.0)
    lazy_bias = -_np_softplus((h @ lm_W3 + lm_b3)[..., 0])
    attn_mask = ninf_mask + np.float32(ALPHA) * lazy_bias
    scores = np.einsum('bhpd,bhnd->bhpn', q, k, dtype=np.float32) / np.float32(np.sqrt(D))
    scores = scores + attn_mask[:, None, :, :]
    scores = scores - scores.max(axis=-1, keepdims=True)
    e = np.exp(scores, dtype=np.float32)
    attn = e / e.sum(axis=-1, keepdims=True)
    out = np.einsum('bhpn,bhnd->bhpd', attn, v, dtype=np.float32)
    out_concat = out.transpose(0, 2, 1, 3).reshape(b, p, H * D)
    mh_out = out_concat @ W_comb + b_comb
    return mh_out.astype(np.float32)


def _build_nc():
    """Device program: pointer score + masked softmax, per core (BPC batches).

    score = CLIP*tanh((mh_out @ nodes^T)/sqrt(E)) + ninf ; probs = softmax_N(score)
    """
    nc = bacc.Bacc("TRN2", target_bir_lowering=False, debug=False,
                   num_devices=NCORES)
    f32 = mybir.dt.float32
    mhT = nc.dram_tensor("mh_outT", [BPC, E, P], f32, kind="ExternalInput").ap()
    ndT = nc.dram_tensor("nodesT", [BPC, E, N], f32, kind="ExternalInput").ap()
    ninf = nc.dram_tensor("ninf", [BPC, P, N], f32, kind="ExternalInput").ap()
    probs = nc.dram_tensor("probs", [BPC, P, N], f32, kind="ExternalOutput").ap()
    NT = 2  # n tiles of 500
    NW = N // NT
    with tile.TileContext(nc) as tc:
        with (
            tc.tile_pool(name="io", bufs=2) as io_pool,
            tc.tile_pool(name="wrk", bufs=2) as wrk_pool,
            tc.tile_pool(name="ps", bufs=2, space="PSUM") as ps_pool,
            tc.tile_pool(name="small", bufs=4) as sm_pool,
        ):
            for b in range(BPC):
                mh_sb = io_pool.tile([E, P], f32, tag="mh")
                nc.sync.dma_start(mh_sb[:], mhT[b])
                nd_sb = io_pool.tile([E, N], f32, tag="nd")
                nc.sync.dma_start(nd_sb[:], ndT[b])
                ninf_sb = io_pool.tile([P, N], f32, tag="ninf")
                nc.sync.dma_start(ninf_sb[:], ninf[b])
                es_tiles = []
                rsums = []
                for j in range(NT):
                    s_ps = ps_pool.tile([P, NW], f32, tag="s")
                    nc.tensor.matmul(s_ps[:], mh_sb[:], nd_sb[:, j * NW:(j + 1) * NW],
                                     start=True, stop=True)
                    t_sb = wrk_pool.tile([P, NW], f32, tag="t")
                    nc.scalar.activation(t_sb[:], s_ps[:],
                                         mybir.ActivationFunctionType.Tanh,
                                         scale=float(1.0 / np.sqrt(E)))
                    # CLIP*tanh + ninf
                    m_sb = wrk_pool.tile([P, NW], f32, tag="m")
                    nc.vector.tensor_scalar(m_sb[:], t_sb[:], CLIP, None,
                                            mybir.AluOpType.mult)
                    a_sb = wrk_pool.tile([P, NW], f32, tag="a")
                    nc.vector.tensor_add(a_sb[:], m_sb[:],
                                         ninf_sb[:, j * NW:(j + 1) * NW])
                    e_sb = wrk_pool.tile([P, NW], f32, tag="e")
                    rs = sm_pool.tile([P, 1], f32, tag="rs")
                    nc.scalar.activation(e_sb[:], a_sb[:],
                                         mybir.ActivationFunctionType.Exp,
                                         accum_out=rs[:])
                    es_tiles.append(e_sb)
                    rsums.append(rs)
                tot = sm_pool.tile([P, 1], f32, tag="tot")
                nc.vector.tensor_add(tot[:], rsums[0][:], rsums[1][:])
                rec = sm_pool.tile([P, 1], f32, tag="rec")
                nc.vector.reciprocal(rec[:], tot[:])
                for j in range(NT):
                    o_sb = wrk_pool.tile([P, NW], f32, tag="o")
                    nc.vector.tensor_scalar(o_sb[:], es_tiles[j][:], rec[:], None,
                                            mybir.AluOpType.mult)
                    nc.sync.dma_start(probs[b, :, j * NW:(j + 1) * NW], o_sb[:])
    nc.compile()
    return nc


_NC_CACHE = None


def kernel(**inputs):
    global _NC_CACHE
    inp = {k: np.asarray(v, dtype=np.float32) for k, v in inputs.items()}
    # host: shard over batch, compute everything up to mh_out per shard
    mh = _host_mh_out(**inp)                                  # (B,P,E)
    mhT = np.ascontiguousarray(mh.transpose(0, 2, 1))         # (B,E,P)
    ndT = np.ascontiguousarray(inp["encoded_nodes"].transpose(0, 2, 1))  # (B,E,N)
    ninf = np.ascontiguousarray(inp["ninf_mask"])
    if _NC_CACHE is None:
        _NC_CACHE = _build_nc()
    nc = _NC_CACHE
    in_maps = []
    for c in range(NCORES):
        s = slice(c * BPC, (c + 1) * BPC)
        in_maps.append({"mh_outT": np.ascontiguousarray(mhT[s]),
                        "nodesT": np.ascontiguousarray(ndT[s]),
                        "ninf": np.ascontiguousarray(ninf[s])})
    res = run_bass_kernel_spmd(nc, in_maps, list(range(NCORES)))
    out = np.concatenate([res.results[c]["probs"] for c in range(NCORES)], axis=0)
    return out.astype(np.float32)



# revision 10
# speedup vs baseline: 5.4746x; 5.4746x over previous
import os
import numpy as np
from ml_dtypes import bfloat16

import concourse.bass as bass
import concourse.tile as tile
from concourse import bacc, mybir
from concourse.bass_utils import run_bass_kernel_spmd

E, H, D = 128, 8, 16
QDIM, DYN, HID = 16, 3, 64
CLIP = 10.0
NCORES = 8
B, P, N = 16, 100, 1000
BPC = B // NCORES          # batches per core
NT = (N + 127) // 128      # n tiles of 128 (last = 104)
LASTN = N - 128 * (NT - 1)

f32 = mybir.dt.float32
bf16 = mybir.dt.bfloat16
AF = mybir.ActivationFunctionType
ALU = mybir.AluOpType


def _build_nc():
    KB = int(os.environ.get("KBISECT", "0"))
    nc = bacc.Bacc("TRN2", target_bir_lowering=False, debug=False,
                   num_devices=NCORES)

    def din(name, shape, dt=f32):
        return nc.dram_tensor(name, shape, dt, kind="ExternalInput").ap()

    xt = din("xt", [BPC, 25, 25, 500], bf16)
    nodes = din("nodes", [BPC, N, E])
    last = din("last", [BPC, P, E])
    loadv = din("loadv", [BPC, 1, P])
    ninf = din("ninf", [BPC, P, N])
    wl1t = din("wl1t", [4, 25, E], bf16)   # L1 templates, row 24 zero (c1 slot)
    w2d = din("w2d", [E, E], bf16)
    w3d = din("w3d", [E, 32], bf16)
    wq2 = din("wq2", [E, E])
    wq2b = din("wq2b", [1, E])
    fw1a = din("fw1a", [E, E])
    fw1b = din("fw1b", [1, E])
    fw1c = din("fw1c", [E, 1])
    fw1d = din("fw1d", [1, 1])
    fw2a = din("fw2a", [E, E])
    fw2b = din("fw2b", [1, E])
    fb1 = din("fb1", [E, 1])
    fb1e = din("fb1e", [1, 1])
    fb2 = din("fb2", [E, 1])
    lqw = din("lqw", [E, QDIM])
    lqwb = din("lqwb", [1, QDIM])
    lqb = din("lqb", [QDIM, 1])
    lmw1q = din("lmw1q", [QDIM, HID])
    lmb1 = din("lmb1", [HID, 1])
    b2b2 = din("b2b2", [E, 1])
    nb3 = din("nb3", [E, 1])
    wk = din("wk", [E, E])
    wv = din("wv", [E, E])
    wcomb = din("wcomb", [E, E])
    bcomb = din("bcomb", [E, 1])
    identm = din("identm", [E, E])
    probs = nc.dram_tensor("probs", [BPC, P, N], f32, kind="ExternalOutput").ap()

    with tile.TileContext(nc) as tc:
        with (
            tc.tile_pool(name="wpool", bufs=1) as wp,
            tc.tile_pool(name="perb", bufs=1) as pb,
            tc.tile_pool(name="sball", bufs=3) as sb,
            tc.tile_pool(name="ps_ptr", bufs=2, space="PSUM") as pptr,
        ):
            def wtile(shape, dt, src, tag):
                t = wp.tile(shape, dt, tag=tag, name=tag)
                nc.sync.dma_start(t[:], src)
                return t

            identd = wtile([E, E], f32, identm, "identd")
            w2d_sb = wtile([E, E], bf16, w2d, "w2d")
            w3d_sb = wtile([E, 32], bf16, w3d, "w3d")
            wq2_sb = wtile([E, E], f32, wq2, "wq2")
            wq2b_sb = wtile([1, E], f32, wq2b, "wq2b")
            fw1a_sb = wtile([E, E], f32, fw1a, "fw1a")
            fw1b_sb = wtile([1, E], f32, fw1b, "fw1b")
            fw1c_sb = wtile([E, 1], f32, fw1c, "fw1c")
            fw1d_sb = wtile([1, 1], f32, fw1d, "fw1d")
            fw2a_sb = wtile([E, E], f32, fw2a, "fw2a")
            fw2b_sb = wtile([1, E], f32, fw2b, "fw2b")
            fb1_sb = wtile([E, 1], f32, fb1, "fb1")
            fb1e_sb = wtile([1, 1], f32, fb1e, "fb1e")
            fb2_sb = wtile([E, 1], f32, fb2, "fb2")
            lqw_sb = wtile([E, QDIM], f32, lqw, "lqw")
            lqwb_sb = wtile([1, QDIM], f32, lqwb, "lqwb")
            lqb_sb = wtile([QDIM, 1], f32, lqb, "lqb")
            lmw1q_sb = wtile([QDIM, HID], f32, lmw1q, "lmw1q")
            lmb1_sb = wtile([HID, 1], f32, lmb1, "lmb1")
            b2b2_sb = wtile([E, 1], f32, b2b2, "b2b2")
            nb3_sb = wtile([E, 1], f32, nb3, "nb3")
            wk_sb = wtile([E, E], f32, wk, "wk")
            wv_sb = wtile([E, E], f32, wv, "wv")
            wcomb_sb = wtile([E, E], f32, wcomb, "wcomb")
            bcomb_sb = wtile([E, 1], f32, bcomb, "bcomb")
            ones_sb = wp.tile([E, 1], f32, tag="ones", name="ones")
            nc.vector.memset(ones_sb[:], 1.0)

            ndT = [pb.tile([E, 1024], f32, tag=f"ndT{b}", name=f"ndT{b}") for b in range(BPC)]
            kTt = [pb.tile([E, 1024], f32, tag=f"kT{b}", name=f"kT{b}") for b in range(BPC)]
            vsb = [pb.tile([E, NT * E], f32, tag=f"v{b}", name=f"v{b}") for b in range(BPC)]
            qTt = [pb.tile([E, P], f32, tag=f"qT{b}", name=f"qT{b}") for b in range(BPC)]
            qho = [[pb.tile([D, P], f32, tag=f"qh{b}{h}", name=f"qh{b}{h}")
                    for h in range(1, H, 2)] for b in range(BPC)]
            kho = [[pb.tile([D, 1024], f32, tag=f"kh{b}{h}", name=f"kh{b}{h}")
                    for h in range(1, H, 2)] for b in range(BPC)]
            Gt = [pb.tile([P, N], f32, tag=f"G{b}", name=f"G{b}") for b in range(BPC)]
            GTt = [pb.tile([E, NT * P], f32, tag=f"GT{b}", name=f"GT{b}") for b in range(BPC)]
            mhT = [pb.tile([E, P], f32, tag=f"mh{b}", name=f"mh{b}") for b in range(BPC)]
            enT = [pb.tile([P, N], f32, tag=f"en{b}", name=f"en{b}") for b in range(BPC)]
            c1T = [pb.tile([P, E], bf16, tag=f"c1T{b}", name=f"c1T{b}") for b in range(BPC)]

            # ============ PHASE 1: prep + lazy-mask MLP ============
            with (
                tc.tile_pool(name="ps_mlp", bufs=3, space="PSUM") as pmlp,
                tc.tile_pool(name="ps_s3", bufs=2, space="PSUM") as ps3,
                tc.tile_pool(name="xtp", bufs=3) as xtp,
                tc.tile_pool(name="w1pp", bufs=3) as w1pp,
                tc.tile_pool(name="mlpact", bufs=3) as ma,
            ):
                for b in range(BPC):
                    nc.vector.memset(ndT[b][:, N:1024], 0.0)
                    nc.vector.memset(kTt[b][:, N:1024], 0.0)
                    # nodes^T
                    for half in range(2):
                        ps = pptr.tile([E, 512], f32, tag="ptr", name="ps")
                        for c in range(4):
                            t = half * 4 + c
                            rows = LASTN if t == NT - 1 else 128
                            nd_in = ma.tile([128, E], f32, tag="ndin", name="nd_in")
                            nc.sync.dma_start(nd_in[:rows, :],
                                              nodes[b, t * 128:t * 128 + rows, :])
                            nc.tensor.transpose(ps[:, c * 128:c * 128 + rows],
                                                nd_in[:rows, :], identd[:rows, :rows])
                        w = 512 if half == 0 else N - 512
                        nc.scalar.copy(ndT[b][:, half * 512:half * 512 + w], ps[:, :w])
                    # kT
                    for half in range(2):
                        ps = pptr.tile([E, 512], f32, tag="ptr", name="ps")
                        nc.tensor.matmul(ps[:, :500], wk_sb[:],
                                         ndT[b][:, half * 500:half * 500 + 500],
                                         start=True, stop=True)
                        nc.scalar.copy(kTt[b][:, half * 500:half * 500 + 500], ps[:, :500])
                    for i, h in enumerate(range(1, H, 2)):
                        nc.sync.dma_start(kho[b][i][:], kTt[b][h * D:(h + 1) * D, :])
                    # v natural
                    for half in range(2):
                        ps = pptr.tile([E, 512], f32, tag="ptr", name="ps")
                        for c in range(4):
                            t = half * 4 + c
                            nc.tensor.matmul(ps[:, c * 128:c * 128 + 128],
                                             ndT[b][:, t * 128:t * 128 + 128],
                                             wv_sb[:], start=True, stop=True)
                        nc.scalar.copy(vsb[b][:, half * 512:half * 512 + 512], ps[:])
                    # q path
                    la_in = ma.tile([P, E], f32, tag="lain", name="la_in")
                    nc.sync.dma_start(la_in[:], last[b])
                    lo_in = ma.tile([1, P], f32, tag="loin", name="lo_in")
                    nc.sync.dma_start(lo_in[:], loadv[b])
                    ps = pptr.tile([E, 512], f32, tag="ptr", name="ps")
                    nc.tensor.transpose(ps[:, :P], la_in[:], identd[:P, :P])
                    laT = ma.tile([E, P], f32, tag="laT", name="laT")
                    nc.scalar.copy(laT[:], ps[:, :P])

                    ps = pptr.tile([E, 512], f32, tag="ptr", name="ps")
                    nc.tensor.matmul(ps[:, :P], fw1a_sb[:], laT[:], start=True, stop=False)
                    nc.tensor.matmul(ps[:, :P], fw1b_sb[:], lo_in[:], start=False, stop=True)
                    r128 = ma.tile([E, P], f32, tag="r128", name="r128")
                    nc.scalar.activation(r128[:], ps[:, :P], AF.Relu, bias=fb1_sb[:, 0:1])
                    ps = pptr.tile([E, 512], f32, tag="ptr", name="ps")
                    nc.tensor.matmul(ps[:1, :P], fw1c_sb[:], laT[:], start=True, stop=False)
                    nc.tensor.matmul(ps[:1, :P], fw1d_sb[:], lo_in[:], start=False, stop=True)
                    rl = ma.tile([1, P], f32, tag="rl", name="rl")
                    nc.scalar.activation(rl[:], ps[:1, :P], AF.Relu, bias=fb1e_sb[:, 0:1])

                    ps = pptr.tile([E, 512], f32, tag="ptr", name="ps")
                    nc.tensor.matmul(ps[:, :P], fw2a_sb[:], r128[:], start=True, stop=False)
                    nc.tensor.matmul(ps[:, :P], fw2b_sb[:], rl[:], start=False, stop=True)
                    sig = ma.tile([E, P], f32, tag="sig", name="sig")
                    nc.scalar.activation(sig[:], ps[:, :P], AF.Sigmoid, bias=fb2_sb[:, 0:1])

                    ps = pptr.tile([E, 512], f32, tag="ptr", name="ps")
                    nc.tensor.matmul(ps[:, :P], wq2_sb[:], laT[:], start=True, stop=False)
                    nc.tensor.matmul(ps[:, :P], wq2b_sb[:], lo_in[:], start=False, stop=True)
                    qraw = ma.tile([E, P], f32, tag="qraw", name="qraw")
                    nc.scalar.copy(qraw[:], ps[:, :P])
                    nc.vector.tensor_mul(qTt[b][:], qraw[:], sig[:])
                    for i, h in enumerate(range(1, H, 2)):
                        nc.sync.dma_start(qho[b][i][:], qTt[b][h * D:(h + 1) * D, :])

                    ps = pptr.tile([E, 512], f32, tag="ptr", name="ps")
                    nc.tensor.matmul(ps[:QDIM, :P], lqw_sb[:], laT[:], start=True, stop=False)
                    nc.tensor.matmul(ps[:QDIM, :P], lqwb_sb[:], lo_in[:], start=False, stop=True)
                    qfT = ma.tile([QDIM, P], f32, tag="qfT", name="qfT")
                    nc.scalar.activation(qfT[:], ps[:QDIM, :P], AF.Identity, bias=lqb_sb[:, 0:1])
                    ps = pptr.tile([E, 512], f32, tag="ptr", name="ps")
                    nc.tensor.matmul(ps[:HID, :P], lmw1q_sb[:], qfT[:], start=True, stop=True)
                    c1s = ma.tile([HID, P], f32, tag="c1s", name="c1s")
                    nc.scalar.activation(c1s[:], ps[:HID, :P], AF.Identity, bias=lmb1_sb[:, 0:1])
                    c1c1 = ma.tile([E, P], f32, tag="c1c1", name="c1c1")
                    nc.sync.dma_start(c1c1[0:HID, :], c1s[:])
                    nc.sync.dma_start(c1c1[HID:E, :], c1s[:])
                    ps = pptr.tile([E, 512], f32, tag="ptr", name="ps")
                    nc.tensor.transpose(ps[:P, :E], c1c1[:], identd[:])
                    nc.scalar.copy(c1T[b][:], ps[:P, :E])

                    # lazy MLP
                    for g in range(25 if KB != 2 else 0):
                        xg = xtp.tile([25, 500], bf16, tag="xg", name="xg")
                        nc.sync.dma_start(xg[:], xt[b, g])
                        s3ps = ps3.tile([E, 512], f32, tag="s3", name="s3ps")
                        for s in range(4):
                            p = 4 * g + s
                            w1p = w1pp.tile([25, E], bf16, tag="w1p", name="w1p")
                            nc.sync.dma_start(w1p[:24, :], wl1t[s, :24, :])
                            nc.sync.dma_start(w1p[24:25, :], c1T[b][p:p + 1, :])
                            h1ps = pmlp.tile([E, 500], f32, tag="mm", name="h1ps")
                            nc.tensor.matmul(h1ps[:], w1p[:], xg[:], start=True, stop=True)
                            h1sb = ma.tile([E, 500], bf16, tag="h1", name="h1sb")
                            if s % 2 == 0:
                                nc.scalar.activation(h1sb[:], h1ps[:], AF.Relu)
                            else:
                                nc.vector.tensor_scalar(h1sb[:], h1ps[:], 0.0, None, ALU.max)
                            h2ps = pmlp.tile([E, 500], f32, tag="mm", name="h2ps")
                            nc.tensor.matmul(h2ps[:], w2d_sb[:], h1sb[:], start=True, stop=True)
                            h2sb = ma.tile([E, 500], bf16, tag="h2", name="h2sb")
                            if s % 2 == 0:
                                nc.vector.tensor_scalar(h2sb[:], h2ps[:], b2b2_sb[:, 0:1],
                                                        0.0, ALU.add, ALU.max)
                            else:
                                nc.scalar.activation(h2sb[:], h2ps[:], AF.Relu,
                                                     bias=b2b2_sb[:, 0:1])
                            nc.tensor.matmul(s3ps[32 * s:32 * s + 32, :500], w3d_sb[:], h2sb[:],
                                             start=True, stop=True,
                                             tile_position=(0, 32 * s))
                        gs = ma.tile([E, 500], f32, tag="gs", name="gs")
                        nc.scalar.activation(gs[:], s3ps[:, :500], AF.Sigmoid, scale=-1.0,
                                             bias=nb3_sb[:, 0:1])
                        for s in range(4):
                            p = 4 * g + s
                            for a in range(2):
                                nc.sync.dma_start(
                                    Gt[b][p:p + 1, a * 500:(a + 1) * 500],
                                    gs[32 * s + a:32 * s + a + 1, :])

            if KB == 1:
                for b in range(BPC):
                    nc.sync.dma_start(probs[b], Gt[b][:])
            # ============ PHASE 2: attention + pointer ============
            with (
                tc.tile_pool(name="ps_big", bufs=2, space="PSUM") as pbig,
                tc.tile_pool(name="ps_sum", bufs=1, space="PSUM") as psum_p,
                tc.tile_pool(name="ps_out", bufs=2, space="PSUM") as pout,
                tc.tile_pool(name="attact", bufs=3) as aa,
            ):
                for b in range(BPC if KB != 1 else 0):
                    if KB == 2:
                        nc.vector.memset(Gt[b][:], 0.5)
                    nin = aa.tile([P, N], f32, tag="nin", name="nin")
                    nc.sync.dma_start(nin[:], ninf[b])
                    nc.scalar.activation(enT[b][:], nin[:], AF.Exp)
                    G2 = aa.tile([P, 1024], f32, tag="G2", name="G2")
                    nc.vector.memset(G2[:, N:1024], 0.0)
                    nc.vector.tensor_mul(G2[:, :N], Gt[b][:], enT[b][:])
                    for half in range(2):
                        ps = pptr.tile([E, 512], f32, tag="ptr", name="ps")
                        for c in range(4):
                            t = half * 4 + c
                            nc.tensor.transpose(ps[:, c * P:(c + 1) * P],
                                                G2[:, t * 128:t * 128 + 128],
                                                identd[:P, :P])
                        nc.scalar.copy(GTt[b][:, half * 4 * P:(half + 1) * 4 * P],
                                       ps[:, :4 * P])

                    ocat = aa.tile([E, P], f32, tag="ocat", name="ocat")
                    if KB in (3,):
                        nc.sync.dma_start(probs[b], Gt[b][:])
                        continue
                    for h in range(H):
                        q_ap = (qTt[b][h * D:(h + 1) * D, :] if h % 2 == 0
                                else qho[b][h // 2][:])
                        k_ap = (kTt[b] if h % 2 == 0 else kho[b][h // 2])
                        k_off = h * D if h % 2 == 0 else 0
                        egs = []
                        sums = psum_p.tile([1, 512], f32, tag="sums", name="sums")
                        ops = pout.tile([D, 512], f32, tag="ops", name="ops")
                        for half in range(2):
                            scps = pbig.tile([E, 4 * P], f32, tag="big", name="scps")
                            tp = (h * D if h % 2 == 0 else 0, 0)
                            for c in range(4):
                                t = half * 4 + c
                                nc.tensor.matmul(
                                    scps[:, c * P:(c + 1) * P],
                                    k_ap[k_off:k_off + D, t * 128:t * 128 + 128],
                                    q_ap, start=True, stop=True, tile_position=tp)
                            et = aa.tile([E, 4 * P], f32, tag="et", name="et")
                            nc.scalar.activation(et[:], scps[:], AF.Exp, scale=0.25)
                            eg = aa.tile([E, 4 * P], f32, tag="eg", name="eg")
                            nc.vector.tensor_mul(eg[:], et[:],
                                                 GTt[b][:, half * 4 * P:(half + 1) * 4 * P])
                            egs.append(eg)
                        for t in range(NT):
                            eg, c = egs[t // 4], t % 4
                            nc.tensor.matmul(sums[:1, :P], ones_sb[:, :],
                                             eg[:, c * P:(c + 1) * P],
                                             start=(t == 0), stop=(t == NT - 1))
                        for t in range(NT):
                            eg, c = egs[t // 4], t % 4
                            nc.tensor.matmul(ops[:D, :P],
                                             vsb[b][:, t * E + h * D:t * E + (h + 1) * D],
                                             eg[:, c * P:(c + 1) * P],
                                             start=(t == 0), stop=(t == NT - 1))
                        rs = aa.tile([1, P], f32, tag="rs", name="rs")
                        nc.vector.reciprocal(rs[:], sums[:1, :P])
                        rb = aa.tile([D, P], f32, tag="rb", name="rb")
                        nc.gpsimd.partition_broadcast(rb[:], rs[:])
                        oh = aa.tile([D, P], f32, tag="oh", name="oh")
                        nc.vector.tensor_mul(oh[:], ops[:D, :P], rb[:])
                        nc.sync.dma_start(ocat[h * D:(h + 1) * D, :], oh[:])

                    if KB in (4,):
                        nc.sync.dma_start(probs[b, :, 0:100], ocat[0:P, :])
                        nc.sync.dma_start(probs[b, :, 100:1000], Gt[b][:, 100:1000])
                        continue
                    ps = pptr.tile([E, 512], f32, tag="ptr", name="ps")
                    nc.tensor.matmul(ps[:, :P], wcomb_sb[:], ocat[:], start=True, stop=True)
                    nc.scalar.activation(mhT[b][:], ps[:, :P], AF.Identity,
                                         bias=bcomb_sb[:, 0:1])

                    if KB in (5,):
                        nc.sync.dma_start(probs[b, :, 0:100], mhT[b][0:P, :])
                        nc.sync.dma_start(probs[b, :, 100:1000], Gt[b][:, 100:1000])
                        continue
                    us = []
                    acc = [aa.tile([P, 1], f32, tag=f"acc{i}", name=f"acc{i}")
                           for i in range(2)]
                    for half in range(2):
                        ptp = pbig.tile([P, 500], f32, tag="big", name="ptp")
                        nc.tensor.matmul(ptp[:], mhT[b][:],
                                         ndT[b][:, half * 500:(half + 1) * 500],
                                         start=True, stop=True)
                        th = aa.tile([P, 500], f32, tag="th", name="th")
                        nc.scalar.activation(th[:], ptp[:], AF.Tanh,
                                             scale=float(1.0 / np.sqrt(E)))
                        e2 = aa.tile([P, 500], f32, tag="e2", name="e2")
                        nc.scalar.activation(e2[:], th[:], AF.Exp, scale=float(CLIP))
                        u = aa.tile([P, 500], f32, tag=f"u{half}", name=f"u{half}")
                        nc.vector.tensor_mul(u[:], e2[:],
                                             enT[b][:, half * 500:(half + 1) * 500])
                        nc.vector.tensor_reduce(acc[half][:, 0:1], u[:],
                                                mybir.AxisListType.X, ALU.add)
                        us.append(u)
                    tot = aa.tile([P, 1], f32, tag="tot", name="tot")
                    nc.vector.tensor_add(tot[:], acc[0][:], acc[1][:])
                    rp = aa.tile([P, 1], f32, tag="rp", name="rp")
                    nc.vector.reciprocal(rp[:], tot[:])
                    for half in range(2):
                        pr = aa.tile([P, 500], f32, tag="pr", name="pr")
                        nc.vector.tensor_scalar(pr[:], us[half][:], rp[:, 0:1], None, ALU.mult)
                        nc.sync.dma_start(probs[b, :, half * 500:(half + 1) * 500], pr[:])

    nc.compile()
    return nc


_NC_CACHE = None


def _prep_weights(inp):
    w = {}
    lm_W1 = inp["lm_W1"]
    wl1t = np.zeros((4, 25, E), np.float32)
    for s in range(4):
        for a in range(2):
            for c in range(DYN):
                wl1t[s, 6 * s + 3 * a + c, 64 * a:64 * a + HID] = lm_W1[c]
    w["wl1t"] = wl1t.astype(bfloat16)
    w2d = np.zeros((E, E), np.float32)
    w2d[:HID, :HID] = inp["lm_W2"]
    w2d[HID:, HID:] = inp["lm_W2"]
    w["w2d"] = w2d.astype(bfloat16)
    w3d = np.zeros((E, 32), np.float32)
    w3d[:HID, 0] = inp["lm_W3"][:, 0]
    w3d[HID:, 1] = inp["lm_W3"][:, 0]
    w["w3d"] = w3d.astype(bfloat16)
    w["wq2"] = 2.0 * inp["Wq_last"][:E]
    w["wq2b"] = 2.0 * inp["Wq_last"][E:E + 1]
    w["fw1a"] = inp["film_W1"][:E, :E]
    w["fw1b"] = inp["film_W1"][E:E + 1, :E]
    w["fw1c"] = inp["film_W1"][:E, E:E + 1]
    w["fw1d"] = inp["film_W1"][E:E + 1, E:E + 1]
    w["fw2a"] = inp["film_W2"][:E]
    w["fw2b"] = inp["film_W2"][E:E + 1]
    w["fb1"] = inp["film_b1"][:E, None]
    w["fb1e"] = inp["film_b1"][E:E + 1, None]
    w["fb2"] = inp["film_b2"][:, None]
    w["lqw"] = inp["lazy_q_W"][:E]
    w["lqwb"] = inp["lazy_q_W"][E:E + 1]
    w["lqb"] = inp["lazy_q_b"][:, None]
    w["lmw1q"] = lm_W1[DYN:]
    w["lmb1"] = inp["lm_b1"][:, None]
    w["b2b2"] = np.concatenate([inp["lm_b2"], inp["lm_b2"]])[:, None]
    w["nb3"] = np.full((E, 1), -float(inp["lm_b3"][0]), np.float32)
    w["wk"] = inp["Wk"]
    w["wv"] = inp["Wv"]
    w["wcomb"] = inp["W_comb"]
    w["bcomb"] = inp["b_comb"][:, None]
    w["identm"] = np.eye(E, dtype=np.float32)
    return {k: np.ascontiguousarray(v) for k, v in w.items()}


def kernel(**inputs):
    global _NC_CACHE
    inp = {k: np.asarray(v, dtype=np.float32) for k, v in inputs.items()}
    dyn = inp["dyn_features"]
    xt = np.empty((B, 25, 25, 500), dtype=bfloat16)
    xt[:, :, :24, :] = (dyn.reshape(B, 25, 4, 2, 500, 3)
                        .transpose(0, 1, 2, 3, 5, 4)
                        .reshape(B, 25, 24, 500).astype(bfloat16))
    xt[:, :, 24, :] = np.float32(1.0)
    weights = _prep_weights(inp)
    if _NC_CACHE is None:
        _NC_CACHE = _build_nc()
    nc = _NC_CACHE
    in_maps = []
    for c in range(NCORES):
        s = slice(c * BPC, (c + 1) * BPC)
        m = {"xt": xt[s], "nodes": inp["encoded_nodes"][s],
             "last": inp["encoded_last_node"][s],
             "loadv": np.ascontiguousarray(inp["load"][s][:, None, :]),
             "ninf": inp["ninf_mask"][s]}
        m.update(weights)
        in_maps.append(m)
    res = run_bass_kernel_spmd(nc, in_maps, list(range(NCORES)))
    out = np.concatenate([res.results[c]["probs"] for c in range(NCORES)], axis=0)
    return out.astype(np.float32)
